# revision 19
# baseline (speedup 1.0000x reference)
"""Self-contained Trainium2 Bass kernel for nn_Encoder_37203006718635.

Strategy: data-parallel over batch B=8 -> one NeuronCore per batch element.
Each core runs the full bidirectional encoder GRU for its batch element
(sequential, gates-on-partitions layout, bf16 matmuls), then all 128 forward
and 128 backward windowed segment scans for that batch element with the 128
window-starts laid along the matmul free dimension.

The device emits the segment hidden states outs_f/outs_b per core; the host
scatters them into the dp_tri upper-triangle output and extracts the final
hidden vector (pure indexing, no FLOPs).
"""

import numpy as np
import ml_dtypes

import concourse.bass as bass
import concourse.tile as tile
from concourse import bacc, mybir
from concourse.masks import make_identity

# Problem constants (hardcoded per the harness contract).
NSEQ, B, V, E, H, S, T = 128, 8, 32000, 256, 256, 256, 32
NCORES = 8
P = 128                  # partition count / chunk size
GC = 6                   # gate chunks (3*256/128)
PAD = 31                 # window clip padding on each side of the sequence
NJ = NSEQ + 2 * PAD      # 190 padded sequence positions

F32 = mybir.dt.float32
BF16 = mybir.dt.bfloat16
I32 = mybir.dt.int32
AF = mybir.ActivationFunctionType
OP = mybir.AluOpType
BF = ml_dtypes.bfloat16

# matmul dtype knob: "bf16" (default) or "f32" (debug/precision reference)
MM_MODE = "bf16"
# fold G_rz into segment PSUM via identity matmuls (PE) instead of a DVE add
SEG_GFOLD = True
# DUMMY: skip compute loops (I/O-calibration build)
DUMMY = False
LIMIT = None

_PROGRAM = None
PHASES = []


def set_mm_mode(mode):
    global MM_MODE, _PROGRAM
    assert mode in ("bf16", "f32")
    if mode != MM_MODE:
        MM_MODE = mode
        _PROGRAM = None


def _np_mdt():
    return BF if MM_MODE == "bf16" else np.float32


def _pad_fill(nc, tile_ap, lo_col, hi_col, nj):
    """Replicate column lo_col into [0, lo_col) and hi_col into (hi_col, nj)
    with doubling copies. tile_ap is [128, mid, nj]."""
    span = 1
    filled = 0
    while filled < lo_col:
        w = min(span, lo_col - filled)
        dst_lo = lo_col - filled - w
        src_lo = lo_col - filled
        nc.vector.tensor_copy(
            tile_ap[:, :, dst_lo : dst_lo + w], tile_ap[:, :, src_lo : src_lo + w]
        )
        filled += w
        span = filled + 1
    span = 1
    filled = 0
    nright = nj - 1 - hi_col
    while filled < nright:
        w = min(span, nright - filled)
        dst_lo = hi_col + 1 + filled
        src_lo = dst_lo - w
        nc.vector.tensor_copy(
            tile_ap[:, :, dst_lo : dst_lo + w], tile_ap[:, :, src_lo : src_lo + w]
        )
        filled += w
        span = filled + 1


def build_program():
    global _PROGRAM
    if _PROGRAM is not None:
        return _PROGRAM

    MDT = BF16 if MM_MODE == "bf16" else F32
    PHASES.clear()

    nc = bacc.Bacc("TRN2", target_bir_lowering=False, debug=False)

    def mark(nm):
        PHASES.append((nm, len(nc.inst_map)))

    # ---- DRAM I/O ----
    xT_d = nc.dram_tensor("xT", [2, P, NSEQ], MDT, kind="ExternalInput")
    wiT_d, whT_d, gfold_d, bhnrow_d, bhncol_d = {}, {}, {}, {}, {}
    for d, nm in ((0, "f"), (1, "r")):
        wiT_d[d] = nc.dram_tensor(f"wiT_{nm}", [E, 3 * H], MDT, kind="ExternalInput")
        whT_d[d] = nc.dram_tensor(f"whT_{nm}", [H, 3 * H], MDT, kind="ExternalInput")
        gfold_d[d] = nc.dram_tensor(f"gfold_{nm}", [P, GC], F32, kind="ExternalInput")
        bhncol_d[d] = nc.dram_tensor(f"bhncol_{nm}", [P, 2], F32, kind="ExternalInput")
    wisT_d = nc.dram_tensor("wisT", [2 * H, 3 * S], MDT, kind="ExternalInput")
    whsT_d = nc.dram_tensor("whsT", [S, 3 * S], MDT, kind="ExternalInput")
    gsfold_d = nc.dram_tensor("gsfold", [P, GC], F32, kind="ExternalInput")
    bhnrows_d = nc.dram_tensor("bhnrows", [1, 2 * P], MDT, kind="ExternalInput")
    h0fT_d = nc.dram_tensor("h0fT", [2, P, NSEQ], F32, kind="ExternalInput")
    h0bT_d = nc.dram_tensor("h0bT", [2, P, NSEQ], F32, kind="ExternalInput")
    outs_f_d = nc.dram_tensor("outs_f", [T, P, 2, NSEQ], MDT, kind="ExternalOutput")
    outs_b_d = nc.dram_tensor("outs_b", [T, P, 2, NSEQ], MDT, kind="ExternalOutput")
    outs_dram = {0: outs_f_d, 1: outs_b_d}

    with tile.TileContext(nc) as tc:
        import contextlib

        with contextlib.ExitStack() as ctx:
            const = ctx.enter_context(tc.tile_pool(name="const", bufs=1))

            # ---- load constants / weights ----
            xT = const.tile([P, 2, NSEQ], MDT)
            nc.gpsimd.dma_start(out=xT[:], in_=xT_d.ap().rearrange("k p t -> p k t"))

            wi_sb, wh_sb, gfold_sb, bhnrow_sb = {}, {}, {}, {}
            for d in (0, 1):
                wi_sb[d] = const.tile([P, 2, 3 * H], MDT, name=f"wi_sb{d}")
                nc.gpsimd.dma_start(
                    out=wi_sb[d][:],
                    in_=wiT_d[d].ap().rearrange("(k p) g -> p k g", p=P),
                )
                wh_sb[d] = const.tile([P, 2, 3 * H], MDT, name=f"wh_sb{d}")
                nc.gpsimd.dma_start(
                    out=wh_sb[d][:],
                    in_=whT_d[d].ap().rearrange("(k p) g -> p k g", p=P),
                )
                gfold_sb[d] = const.tile([P, GC], F32, name=f"gfold_sb{d}")
                nc.gpsimd.dma_start(out=gfold_sb[d][:], in_=gfold_d[d].ap())
                bhnrow_sb[d] = const.tile([P, 2], F32, name=f"bhncol_sb{d}")
                nc.gpsimd.dma_start(out=bhnrow_sb[d][:], in_=bhncol_d[d].ap())

            wis_sb = const.tile([P, 4, 3 * S], MDT)
            nc.gpsimd.dma_start(
                out=wis_sb[:], in_=wisT_d.ap().rearrange("(k p) g -> p k g", p=P)
            )
            whs_sb = const.tile([P, 2, 3 * S], MDT)
            nc.gpsimd.dma_start(
                out=whs_sb[:], in_=whsT_d.ap().rearrange("(k p) g -> p k g", p=P)
            )
            gsfold_sb = const.tile([P, GC], F32)
            nc.gpsimd.dma_start(out=gsfold_sb[:], in_=gsfold_d.ap())
            bhnrows_sb = const.tile([1, 2 * P], MDT)
            nc.gpsimd.dma_start(out=bhnrows_sb[:], in_=bhnrows_d.ap())

            ident = const.tile([P, P], MDT)
            make_identity(nc, ident[:])
            onesN = const.tile([1, NSEQ], MDT)
            nc.vector.memset(onesN[:], 1.0)
            zero2_sb = const.tile([P, 2], MDT)
            nc.vector.memset(zero2_sb[:], 0.0)

            # outputs (transposed, padded): col l = seq pos j + PAD
            houtT = {}
            houtT[0] = const.tile([P, 2, NJ], MDT, name="houtT0")
            houtT[1] = const.tile([P, 2, NJ], MDT, name="houtT1")
            if DUMMY:
                nc.vector.memset(houtT[0][:], 0.0)
                nc.vector.memset(houtT[1][:], 0.0)

            mark("setup")

            # ---- encoder gi precompute: gi[d][:, c, t] (+bias folds) ----
            gi = {}
            with tc.tile_pool(name="gips", bufs=2, space="PSUM") as gp:
                for d in (0, 1):
                    gi[d] = const.tile([P, GC, NSEQ], F32, name=f"gi{d}")
                    for c in range(GC):
                        ps = gp.tile([P, NSEQ], F32)
                        for k in range(2):
                            nc.tensor.matmul(
                                ps[:],
                                lhsT=wi_sb[d][:, k, c * P : (c + 1) * P],
                                rhs=xT[:, k, :],
                                start=(k == 0),
                                stop=(k == 1),
                            )
                        nc.scalar.activation(
                            gi[d][:, c, :],
                            ps[:],
                            AF.Identity,
                            bias=gfold_sb[d][:, c : c + 1],
                        )
            mark("gi")

            # ---- encoder recurrence (bf16 state lives in houtT cols) ----
            with (
                tc.tile_pool(name="encps0", bufs=2, space="PSUM") as eps0,
                tc.tile_pool(name="encps1", bufs=2, space="PSUM") as eps1,
                tc.tile_pool(name="encew", bufs=4) as ew,
            ):
                epools = {0: eps0, 1: eps1}
                for t in range(0 if DUMMY else NSEQ):
                    for d in (0, 1):
                        j = t if d == 0 else NSEQ - 1 - t
                        jp = j - 1 if d == 0 else j + 1
                        hprev = (
                            zero2_sb[:, :] if t == 0 else houtT[d][:, :, PAD + jp]
                        )
                        ps = epools[d].tile([P, 4], F32, tag="ps")
                        psn = epools[d].tile([P, 2], F32, tag="psn")
                        for c in range(GC):
                            dst = ps[:, c : c + 1] if c < 4 else psn[:, c - 4 : c - 3]
                            for k in range(2):
                                nc.tensor.matmul(
                                    dst,
                                    lhsT=wh_sb[d][:, k, c * P : (c + 1) * P],
                                    rhs=hprev[:, k : k + 1],
                                    start=(k == 0),
                                    stop=(k == 1),
                                )
                        grz = ew.tile([P, 4], F32, tag="grz")
                        nc.vector.tensor_add(grz[:], ps[:, 0:4], gi[d][:, 0:4, j])
                        rz = ew.tile([P, 4], MDT, tag="rz")
                        nc.scalar.activation(rz[:], grz[:], AF.Sigmoid)
                        a = ew.tile([P, 2], F32, tag="a")
                        for cc in range(2):
                            nc.vector.scalar_tensor_tensor(
                                out=a[:, cc : cc + 1],
                                in0=psn[:, cc : cc + 1],
                                scalar=bhnrow_sb[d][:, cc : cc + 1],
                                in1=rz[:, cc : cc + 1],
                                op0=OP.add,
                                op1=OP.mult,
                            )
                        s = ew.tile([P, 2], F32, tag="s")
                        nc.vector.tensor_add(s[:], a[:], gi[d][:, 4:6, j])
                        n = ew.tile([P, 2], MDT, tag="n")
                        nc.scalar.activation(n[:], s[:], AF.Tanh)
                        zc = ew.tile([P, 2], MDT, tag="zc")
                        nc.vector.tensor_scalar(
                            out=zc[:], in0=rz[:, 2:4], scalar1=-1.0, scalar2=1.0,
                            op0=OP.mult, op1=OP.add,
                        )
                        v = ew.tile([P, 2], MDT, tag="v")
                        nc.gpsimd.tensor_mul(v[:], hprev[:, :], rz[:, 2:4])
                        u = ew.tile([P, 2], MDT, tag="u")
                        nc.gpsimd.tensor_mul(u[:], n[:], zc[:])
                        nc.vector.tensor_add(houtT[d][:, :, PAD + j], u[:], v[:])
            mark("encoder")

            # ---- G precompute ----
            # rz part in MDT (feeds PE identity-fold), n part in F32
            G_rz = const.tile([P, 4, NJ], MDT)
            G_n = const.tile([P, 2, NJ], F32)
            if DUMMY:
                nc.vector.memset(G_rz[:], 0.0)
                nc.vector.memset(G_n[:], 0.0)
            with tc.tile_pool(name="gsps", bufs=2, space="PSUM") as gsp:
                for c in range(0 if DUMMY else GC):
                    ps = gsp.tile([P, NSEQ], F32)
                    for k in range(4):
                        rhs = (
                            houtT[0][:, k, PAD : PAD + NSEQ]
                            if k < 2
                            else houtT[1][:, k - 2, PAD : PAD + NSEQ]
                        )
                        nc.tensor.matmul(
                            ps[:],
                            lhsT=wis_sb[:, k, c * P : (c + 1) * P],
                            rhs=rhs,
                            start=(k == 0),
                            stop=(k == 3),
                        )
                    dst = (
                        G_rz[:, c, PAD : PAD + NSEQ]
                        if c < 4
                        else G_n[:, c - 4, PAD : PAD + NSEQ]
                    )
                    nc.scalar.activation(
                        dst, ps[:], AF.Identity, bias=gsfold_sb[:, c : c + 1]
                    )
            if not DUMMY:
                _pad_fill(nc, G_rz, PAD, PAD + NSEQ - 1, NJ)
                _pad_fill(nc, G_n, PAD, PAD + NSEQ - 1, NJ)
            mark("G")

            # ---- segment scans ----
            hseg32, hseg16, outs_sb = {}, {}, {}
            for d, h0d in ((0, h0fT_d), (1, h0bT_d)):
                # f32 recurrence state (slot t%2 = state after step t)
                hseg32[d] = [
                    const.tile([P, 2, NSEQ], F32, name=f"hseg32{d}_0"),
                    const.tile([P, 2, NSEQ], F32, name=f"hseg32{d}_1"),
                ]
                nc.gpsimd.dma_start(
                    out=hseg32[d][1][:], in_=h0d.ap().rearrange("c p w -> p c w")
                )
                hseg16[d] = const.tile([P, 2, NSEQ], MDT, name=f"hseg16{d}")
                nc.vector.tensor_copy(hseg16[d][:], hseg32[d][1][:])
                outs_sb[d] = const.tile([P, T, 2, NSEQ], MDT, name=f"outs_sb{d}")
                if DUMMY:
                    nc.vector.memset(outs_sb[d][:], 0.0)

            with (
                tc.tile_pool(name="segps0", bufs=2, space="PSUM") as sps0,
                tc.tile_pool(name="segps1", bufs=2, space="PSUM") as sps1,
                tc.tile_pool(name="segew", bufs=3) as sew,
            ):
                spools = {0: sps0, 1: sps1}
                for t in range(0 if DUMMY else T):
                    for d in (0, 1):
                        lo = PAD + t if d == 0 else PAD - t
                        hrhs = hseg16[d][:] if t == 0 else outs_sb[d][:, t - 1]
                        ps = spools[d].tile([P, GC, NSEQ], F32, tag="ps")
                        for c in range(GC):
                            last_extra = (c < 4 and SEG_GFOLD) or c >= 4
                            for k in range(2):
                                nc.tensor.matmul(
                                    ps[:, c, :],
                                    lhsT=whs_sb[:, k, c * P : (c + 1) * P],
                                    rhs=hrhs[:, k, :],
                                    start=(k == 0),
                                    stop=(k == 1 and not last_extra),
                                )
                            if c < 4:
                                if SEG_GFOLD:
                                    nc.tensor.matmul(
                                        ps[:, c, :],
                                        lhsT=ident[:],
                                        rhs=G_rz[:, c, lo : lo + NSEQ],
                                        start=False,
                                        stop=True,
                                    )
                            else:
                                cc = c - 4
                                nc.tensor.matmul(
                                    ps[:, c, :],
                                    lhsT=bhnrows_sb[:, cc * P : (cc + 1) * P],
                                    rhs=onesN[:],
                                    start=False,
                                    stop=True,
                                )
                        if SEG_GFOLD:
                            rz = sew.tile([P, 4, NSEQ], MDT, tag="rz")
                            nc.scalar.activation(rz[:], ps[:, 0:4, :], AF.Sigmoid)
                        else:
                            grz = sew.tile([P, 4, NSEQ], F32, tag="grz")
                            nc.vector.tensor_add(
                                grz[:], ps[:, 0:4, :], G_rz[:, :, lo : lo + NSEQ]
                            )
                            rz = sew.tile([P, 4, NSEQ], MDT, tag="rz")
                            nc.scalar.activation(rz[:], grz[:], AF.Sigmoid)
                        a = sew.tile([P, 2, NSEQ], F32, tag="a")
                        nc.vector.tensor_mul(a[:], ps[:, 4:6, :], rz[:, 0:2, :])
                        s = sew.tile([P, 2, NSEQ], F32, tag="s")
                        nc.gpsimd.tensor_add(s[:], a[:], G_n[:, :, lo : lo + NSEQ])
                        n = sew.tile([P, 2, NSEQ], F32, tag="n")
                        nc.scalar.activation(n[:], s[:], AF.Tanh)
                        hprev32 = hseg32[d][(t - 1) % 2][:]
                        hcur32 = hseg32[d][t % 2][:]
                        d1 = sew.tile([P, 2, NSEQ], F32, tag="d1")
                        nc.gpsimd.tensor_sub(d1[:], hprev32, n[:])
                        e = sew.tile([P, 2, NSEQ], F32, tag="e")
                        nc.vector.tensor_mul(e[:], d1[:], rz[:, 2:4, :])
                        nc.vector.tensor_add(hcur32, n[:], e[:])
                        nc.gpsimd.tensor_add(outs_sb[d][:, t], n[:], e[:])
                for d, eng in ((0, nc.sync), (1, nc.scalar)):
                    eng.dma_start(
                        out=outs_dram[d].ap().rearrange("t p c w -> p t c w"),
                        in_=outs_sb[d][:],
                    )
            mark("segments")

    nc.compile()
    mark("tail")
    _PROGRAM = nc
    return nc


def _prep_in_maps(tokens, emb, Wi_f, Wh_f, bi_f, bh_f, Wi_r, Wh_r, bi_r, bh_r,
                  Wi_s, Wh_s, bi_s, bh_s, h0f, h0b):
    mdt = _np_mdt()

    def gfold(bi, bh):
        v = np.concatenate([(bi + bh)[: 2 * H], bi[2 * H :]]).astype(np.float32)
        return np.ascontiguousarray(v.reshape(GC, P).T)

    def bhnrow(bh):
        return np.ascontiguousarray(
            bh[2 * H :].astype(np.float32).reshape(1, 2 * P)
        ).astype(mdt)

    common = {
        "wiT_f": np.ascontiguousarray(Wi_f.T).astype(mdt),
        "whT_f": np.ascontiguousarray(Wh_f.T).astype(mdt),
        "wiT_r": np.ascontiguousarray(Wi_r.T).astype(mdt),
        "whT_r": np.ascontiguousarray(Wh_r.T).astype(mdt),
        "gfold_f": gfold(bi_f, bh_f),
        "gfold_r": gfold(bi_r, bh_r),
        "bhncol_f": np.ascontiguousarray(bh_f[2 * H :].astype(np.float32).reshape(2, P).T),
        "bhncol_r": np.ascontiguousarray(bh_r[2 * H :].astype(np.float32).reshape(2, P).T),
        "wisT": np.ascontiguousarray(Wi_s.T).astype(mdt),
        "whsT": np.ascontiguousarray(Wh_s.T).astype(mdt),
        "gsfold": gfold(bi_s, bh_s),
        "bhnrows": bhnrow(bh_s),
    }
    in_maps = []
    for p in range(NCORES):
        m = dict(common)
        x = emb[tokens[:, p]]  # (128, 256) host-side embedding gather (indexing)
        m["xT"] = np.ascontiguousarray(
            x.T.astype(np.float32).reshape(2, P, NSEQ)
        ).astype(mdt)
        m["h0fT"] = np.ascontiguousarray(
            h0f[:, p, :].T.astype(np.float32).reshape(2, P, NSEQ)
        )
        m["h0bT"] = np.ascontiguousarray(
            h0b[:, p, :].T.astype(np.float32).reshape(2, P, NSEQ)
        )
        in_maps.append(m)
    return in_maps


_TRI_IDX = None


def _tri_indices():
    global _TRI_IDX
    if _TRI_IDX is not None:
        return _TRI_IDX
    off = np.zeros(NSEQ, dtype=np.int64)
    for r in range(1, NSEQ):
        off[r] = off[r - 1] + (NSEQ - (r - 1))
    fk, fi, ft = [], [], []
    for i in range(NSEQ):
        L = min(T, NSEQ - i)
        t = np.arange(L)
        fk.append(off[i] + t)
        fi.append(np.full(L, i))
        ft.append(t)
    bk, bi_, bt = [], [], []
    for i in range(NSEQ):
        L = min(T, i + 1)
        t = np.arange(L)
        bk.append(off[i - t] + t)
        bi_.append(np.full(L, i))
        bt.append(t)
    _TRI_IDX = (
        off,
        np.concatenate(fk), np.concatenate(fi), np.concatenate(ft),
        np.concatenate(bk), np.concatenate(bi_), np.concatenate(bt),
    )
    return _TRI_IDX


_RUNNER = None


def _get_runner():
    """Build the sharded jit callable once and cache it (run_bass_via_pjrt
    rebuilds its closure per call, which forces a jax retrace every time)."""
    global _RUNNER
    if _RUNNER is not None:
        return _RUNNER
    import jax
    from concourse import bass2jax

    bass2jax.install_neuronx_cc_hook()
    nc = build_program()
    pname = nc.partition_id_tensor.name if nc.partition_id_tensor else None
    in_names, out_names, out_avals, zero_shapes = [], [], [], []
    for alloc in nc.m.functions[0].allocations:
        if not isinstance(alloc, mybir.MemoryLocationSet):
            continue
        name = alloc.memorylocations[0].name
        if alloc.kind == "ExternalInput":
            if name != pname:
                in_names.append(name)
        elif alloc.kind == "ExternalOutput":
            out_names.append(name)
            shape = tuple(alloc.tensor_shape)
            dtype = mybir.dt.np(alloc.dtype)
            out_avals.append(jax.core.ShapedArray(shape, dtype))
            zero_shapes.append((shape, dtype))
    n_params = len(in_names)
    all_names = in_names + out_names
    if pname is not None:
        all_names = all_names + [pname]
    donate = tuple(range(n_params, n_params + len(out_names)))

    def _body(*args):
        operands = list(args)
        if pname is not None:
            operands.append(bass2jax.partition_id_tensor())
        outs = bass2jax._bass_exec_p.bind(
            *operands,
            out_avals=tuple(out_avals),
            in_names=tuple(all_names),
            out_names=tuple(out_names),
            lowering_input_output_aliases=(),
            sim_require_finite=True,
            sim_require_nnan=True,
            nc=nc,
        )
        return tuple(outs)

    devices = jax.devices()[:NCORES]
    mesh = bass2jax.Mesh(np.asarray(devices), ("core",))
    in_specs = (bass2jax.PartitionSpec("core"),) * (n_params + len(out_names))
    out_specs = (bass2jax.PartitionSpec("core"),) * len(out_names)
    sharded = jax.jit(
        bass2jax.shard_map(
            _body, mesh=mesh, in_specs=in_specs, out_specs=out_specs, check_rep=False
        ),
        keep_unused=True,
    )
    _RUNNER = (sharded, in_names, out_names, out_avals, zero_shapes, mesh)
    return _RUNNER


_DEV_IN_CACHE = {}


def run_on_device(in_maps):
    import jax
    from jax.sharding import NamedSharding

    sharded, in_names, out_names, out_avals, zero_shapes, mesh = _get_runner()
    from jax.sharding import PartitionSpec

    spec = NamedSharding(mesh, PartitionSpec("core"))
    concat_in = [
        np.concatenate([np.asarray(in_maps[c][nm]) for c in range(NCORES)], axis=0)
        for nm in in_names
    ]
    placed = [jax.device_put(a, spec) for a in concat_in]
    zeros = _DEV_IN_CACHE.get("zeros")
    if zeros is None:
        zeros = [
            jax.device_put(np.zeros((NCORES * sh[0], *sh[1:]), dt), spec)
            for sh, dt in zero_shapes
        ]
        _DEV_IN_CACHE["zeros"] = zeros
    out_arrs = sharded(*placed, *zeros)
    results = [
        {
            nm: np.asarray(out_arrs[i]).reshape(NCORES, *out_avals[i].shape)[c]
            for i, nm in enumerate(out_names)
        }
        for c in range(NCORES)
    ]

    class _R:
        pass

    r = _R()
    r.results = results
    return r


def assemble(results):
    def _tw(arr):  # (T, P, 2, W) -> (T, W, S) with s = c*128 + p
        return arr.astype(np.float32).transpose(0, 3, 2, 1).reshape(T, NSEQ, 2 * P)
    OF = np.stack([_tw(results[p]["outs_f"]) for p in range(NCORES)])
    OB = np.stack([_tw(results[p]["outs_b"]) for p in range(NCORES)])
    _, fk, fi, ft, bk, bi_, bt = _tri_indices()
    ntri = NSEQ * (NSEQ + 1) // 2
    dp_tri = np.zeros((ntri, B, 2 * S), dtype=np.float32)
    dp_tri[fk, :, :S] = OF[:, ft, fi, :].transpose(1, 0, 2)
    dp_tri[bk, :, S:] = OB[:, bt, bi_, :].transpose(1, 0, 2)
    hid_f = OF[:, 0, NSEQ - 1, :]
    hid_b = OB[:, T - 1, NSEQ - 1, :]
    hidden = np.concatenate([hid_f, hid_b], axis=-1)[None]
    return dp_tri, hidden


def kernel(**inputs):
    in_maps = _prep_in_maps(**inputs)
    res = run_on_device(in_maps)
    return assemble(res.results)


# revision 20
# speedup vs baseline: 1.0209x; 1.0209x over previous
"""Self-contained Trainium2 Bass kernel for nn_Encoder_37203006718635.

Strategy: data-parallel over batch B=8 -> one NeuronCore per batch element.
Each core runs the full bidirectional encoder GRU for its batch element
(sequential, gates-on-partitions layout, bf16 matmuls), then all 128 forward
and 128 backward windowed segment scans for that batch element with the 128
window-starts laid along the matmul free dimension.

The device emits the segment hidden states outs_f/outs_b per core; the host
scatters them into the dp_tri upper-triangle output and extracts the final
hidden vector (pure indexing, no FLOPs).
"""

import numpy as np
import ml_dtypes

import concourse.bass as bass
import concourse.tile as tile
from concourse import bacc, mybir
from concourse.masks import make_identity

# Problem constants (hardcoded per the harness contract).
NSEQ, B, V, E, H, S, T = 128, 8, 32000, 256, 256, 256, 32
NCORES = 8
P = 128                  # partition count / chunk size
GC = 6                   # gate chunks (3*256/128)
PAD = 31                 # window clip padding on each side of the sequence
NJ = NSEQ + 2 * PAD      # 190 padded sequence positions

F32 = mybir.dt.float32
BF16 = mybir.dt.bfloat16
I32 = mybir.dt.int32
AF = mybir.ActivationFunctionType
OP = mybir.AluOpType
BF = ml_dtypes.bfloat16

# matmul dtype knob: "bf16" (default) or "f32" (debug/precision reference)
MM_MODE = "bf16"
# fold G_rz into segment PSUM via identity matmuls (PE) instead of a DVE add
SEG_GFOLD = True
# DUMMY: skip compute loops (I/O-calibration build)
DUMMY = False
LIMIT = None

_PROGRAM = None
PHASES = []


def set_mm_mode(mode):
    global MM_MODE, _PROGRAM
    assert mode in ("bf16", "f32")
    if mode != MM_MODE:
        MM_MODE = mode
        _PROGRAM = None


def _np_mdt():
    return BF if MM_MODE == "bf16" else np.float32


def _pad_fill(nc, tile_ap, lo_col, hi_col, nj):
    """Replicate column lo_col into [0, lo_col) and hi_col into (hi_col, nj)
    with doubling copies. tile_ap is [128, mid, nj]."""
    span = 1
    filled = 0
    while filled < lo_col:
        w = min(span, lo_col - filled)
        dst_lo = lo_col - filled - w
        src_lo = lo_col - filled
        nc.vector.tensor_copy(
            tile_ap[:, :, dst_lo : dst_lo + w], tile_ap[:, :, src_lo : src_lo + w]
        )
        filled += w
        span = filled + 1
    span = 1
    filled = 0
    nright = nj - 1 - hi_col
    while filled < nright:
        w = min(span, nright - filled)
        dst_lo = hi_col + 1 + filled
        src_lo = dst_lo - w
        nc.vector.tensor_copy(
            tile_ap[:, :, dst_lo : dst_lo + w], tile_ap[:, :, src_lo : src_lo + w]
        )
        filled += w
        span = filled + 1


def build_program():
    global _PROGRAM
    if _PROGRAM is not None:
        return _PROGRAM

    MDT = BF16 if MM_MODE == "bf16" else F32
    PHASES.clear()

    nc = bacc.Bacc("TRN2", target_bir_lowering=False, debug=False)

    def mark(nm):
        PHASES.append((nm, len(nc.inst_map)))

    # ---- DRAM I/O ----
    xT_d = nc.dram_tensor("xT", [2, P, NSEQ], MDT, kind="ExternalInput")
    wiT_d, whT_d, gfold_d, bhnrow_d, bhncol_d = {}, {}, {}, {}, {}
    for d, nm in ((0, "f"), (1, "r")):
        wiT_d[d] = nc.dram_tensor(f"wiT_{nm}", [E, 3 * H], MDT, kind="ExternalInput")
        whT_d[d] = nc.dram_tensor(f"whT_{nm}", [H, 3 * H], MDT, kind="ExternalInput")
        gfold_d[d] = nc.dram_tensor(f"gfold_{nm}", [P, GC], F32, kind="ExternalInput")
        bhncol_d[d] = nc.dram_tensor(f"bhncol_{nm}", [P, 2], F32, kind="ExternalInput")
    wisT_d = nc.dram_tensor("wisT", [2 * H, 3 * S], MDT, kind="ExternalInput")
    whsT_d = nc.dram_tensor("whsT", [S, 3 * S], MDT, kind="ExternalInput")
    gsfold_d = nc.dram_tensor("gsfold", [P, GC], F32, kind="ExternalInput")
    bhnrows_d = nc.dram_tensor("bhnrows", [P, 2], F32, kind="ExternalInput")
    h0fT_d = nc.dram_tensor("h0fT", [2, P, NSEQ], F32, kind="ExternalInput")
    h0bT_d = nc.dram_tensor("h0bT", [2, P, NSEQ], F32, kind="ExternalInput")
    outs_f_d = nc.dram_tensor("outs_f", [T, P, 2, NSEQ], MDT, kind="ExternalOutput")
    outs_b_d = nc.dram_tensor("outs_b", [T, P, 2, NSEQ], MDT, kind="ExternalOutput")
    outs_dram = {0: outs_f_d, 1: outs_b_d}

    with tile.TileContext(nc) as tc:
        import contextlib

        with contextlib.ExitStack() as ctx:
            const = ctx.enter_context(tc.tile_pool(name="const", bufs=1))

            # ---- load constants / weights ----
            xT = const.tile([P, 2, NSEQ], MDT)
            nc.gpsimd.dma_start(out=xT[:], in_=xT_d.ap().rearrange("k p t -> p k t"))

            wi_sb, wh_sb, gfold_sb, bhnrow_sb = {}, {}, {}, {}
            for d in (0, 1):
                wi_sb[d] = const.tile([P, 2, 3 * H], MDT, name=f"wi_sb{d}")
                nc.gpsimd.dma_start(
                    out=wi_sb[d][:],
                    in_=wiT_d[d].ap().rearrange("(k p) g -> p k g", p=P),
                )
                wh_sb[d] = const.tile([P, 2, 3 * H], MDT, name=f"wh_sb{d}")
                nc.gpsimd.dma_start(
                    out=wh_sb[d][:],
                    in_=whT_d[d].ap().rearrange("(k p) g -> p k g", p=P),
                )
                gfold_sb[d] = const.tile([P, GC], F32, name=f"gfold_sb{d}")
                nc.gpsimd.dma_start(out=gfold_sb[d][:], in_=gfold_d[d].ap())
                bhnrow_sb[d] = const.tile([P, 2], F32, name=f"bhncol_sb{d}")
                nc.gpsimd.dma_start(out=bhnrow_sb[d][:], in_=bhncol_d[d].ap())

            wis_sb = const.tile([P, 4, 3 * S], MDT)
            nc.gpsimd.dma_start(
                out=wis_sb[:], in_=wisT_d.ap().rearrange("(k p) g -> p k g", p=P)
            )
            whs_sb = const.tile([P, 2, 3 * S], MDT)
            nc.gpsimd.dma_start(
                out=whs_sb[:], in_=whsT_d.ap().rearrange("(k p) g -> p k g", p=P)
            )
            gsfold_sb = const.tile([P, GC], F32)
            nc.gpsimd.dma_start(out=gsfold_sb[:], in_=gsfold_d.ap())
            bhnscol_sb = const.tile([P, 2], F32)
            nc.gpsimd.dma_start(out=bhnscol_sb[:], in_=bhnrows_d.ap())

            ident = const.tile([P, P], MDT)
            make_identity(nc, ident[:])
            onesN = const.tile([1, NSEQ], MDT)
            nc.vector.memset(onesN[:], 1.0)
            zero2_sb = const.tile([P, 2], MDT)
            nc.vector.memset(zero2_sb[:], 0.0)

            # outputs (transposed, padded): col l = seq pos j + PAD
            houtT = {}
            houtT[0] = const.tile([P, 2, NJ], MDT, name="houtT0")
            houtT[1] = const.tile([P, 2, NJ], MDT, name="houtT1")
            if DUMMY:
                nc.vector.memset(houtT[0][:], 0.0)
                nc.vector.memset(houtT[1][:], 0.0)

            mark("setup")

            # ---- encoder gi precompute: gi[d][:, c, t] (+bias folds) ----
            gi = {}
            with tc.tile_pool(name="gips", bufs=2, space="PSUM") as gp:
                for d in (0, 1):
                    gi[d] = const.tile([P, GC, NSEQ], F32, name=f"gi{d}")
                    for c in range(GC):
                        ps = gp.tile([P, NSEQ], F32)
                        for k in range(2):
                            nc.tensor.matmul(
                                ps[:],
                                lhsT=wi_sb[d][:, k, c * P : (c + 1) * P],
                                rhs=xT[:, k, :],
                                start=(k == 0),
                                stop=(k == 1),
                            )
                        nc.scalar.activation(
                            gi[d][:, c, :],
                            ps[:],
                            AF.Identity,
                            bias=gfold_sb[d][:, c : c + 1],
                        )
            mark("gi")

            # ---- encoder recurrence (bf16 state lives in houtT cols) ----
            with (
                tc.tile_pool(name="encps0", bufs=2, space="PSUM") as eps0,
                tc.tile_pool(name="encps1", bufs=2, space="PSUM") as eps1,
                tc.tile_pool(name="encew", bufs=4) as ew,
            ):
                epools = {0: eps0, 1: eps1}
                for t in range(0 if DUMMY else NSEQ):
                    for d in (0, 1):
                        j = t if d == 0 else NSEQ - 1 - t
                        jp = j - 1 if d == 0 else j + 1
                        hprev = (
                            zero2_sb[:, :] if t == 0 else houtT[d][:, :, PAD + jp]
                        )
                        ps = epools[d].tile([P, 4], F32, tag="ps")
                        psn = epools[d].tile([P, 2], F32, tag="psn")
                        for c in range(GC):
                            dst = ps[:, c : c + 1] if c < 4 else psn[:, c - 4 : c - 3]
                            for k in range(2):
                                nc.tensor.matmul(
                                    dst,
                                    lhsT=wh_sb[d][:, k, c * P : (c + 1) * P],
                                    rhs=hprev[:, k : k + 1],
                                    start=(k == 0),
                                    stop=(k == 1),
                                )
                        grz = ew.tile([P, 4], F32, tag="grz")
                        nc.vector.tensor_add(grz[:], ps[:, 0:4], gi[d][:, 0:4, j])
                        rz = ew.tile([P, 4], MDT, tag="rz")
                        nc.scalar.activation(rz[:], grz[:], AF.Sigmoid)
                        a = ew.tile([P, 2], F32, tag="a")
                        for cc in range(2):
                            nc.vector.scalar_tensor_tensor(
                                out=a[:, cc : cc + 1],
                                in0=psn[:, cc : cc + 1],
                                scalar=bhnrow_sb[d][:, cc : cc + 1],
                                in1=rz[:, cc : cc + 1],
                                op0=OP.add,
                                op1=OP.mult,
                            )
                        s = ew.tile([P, 2], F32, tag="s")
                        nc.vector.tensor_add(s[:], a[:], gi[d][:, 4:6, j])
                        n = ew.tile([P, 2], MDT, tag="n")
                        nc.scalar.activation(n[:], s[:], AF.Tanh)
                        zc = ew.tile([P, 2], MDT, tag="zc")
                        nc.vector.tensor_scalar(
                            out=zc[:], in0=rz[:, 2:4], scalar1=-1.0, scalar2=1.0,
                            op0=OP.mult, op1=OP.add,
                        )
                        v = ew.tile([P, 2], MDT, tag="v")
                        nc.gpsimd.tensor_mul(v[:], hprev[:, :], rz[:, 2:4])
                        u = ew.tile([P, 2], MDT, tag="u")
                        nc.gpsimd.tensor_mul(u[:], n[:], zc[:])
                        nc.vector.tensor_add(houtT[d][:, :, PAD + j], u[:], v[:])
            mark("encoder")

            # ---- G precompute ----
            # rz part in MDT (feeds PE identity-fold), n part in F32
            G_rz = const.tile([P, 4, NJ], MDT)
            G_n = const.tile([P, 2, NJ], F32)
            if DUMMY:
                nc.vector.memset(G_rz[:], 0.0)
                nc.vector.memset(G_n[:], 0.0)
            with tc.tile_pool(name="gsps", bufs=2, space="PSUM") as gsp:
                for c in range(0 if DUMMY else GC):
                    ps = gsp.tile([P, NSEQ], F32)
                    for k in range(4):
                        rhs = (
                            houtT[0][:, k, PAD : PAD + NSEQ]
                            if k < 2
                            else houtT[1][:, k - 2, PAD : PAD + NSEQ]
                        )
                        nc.tensor.matmul(
                            ps[:],
                            lhsT=wis_sb[:, k, c * P : (c + 1) * P],
                            rhs=rhs,
                            start=(k == 0),
                            stop=(k == 3),
                        )
                    dst = (
                        G_rz[:, c, PAD : PAD + NSEQ]
                        if c < 4
                        else G_n[:, c - 4, PAD : PAD + NSEQ]
                    )
                    nc.scalar.activation(
                        dst, ps[:], AF.Identity, bias=gsfold_sb[:, c : c + 1]
                    )
            if not DUMMY:
                _pad_fill(nc, G_rz, PAD, PAD + NSEQ - 1, NJ)
                _pad_fill(nc, G_n, PAD, PAD + NSEQ - 1, NJ)
            mark("G")

            # ---- segment scans ----
            hseg32, hseg16, outs_sb = {}, {}, {}
            for d, h0d in ((0, h0fT_d), (1, h0bT_d)):
                # f32 recurrence state (slot t%2 = state after step t)
                hseg32[d] = [
                    const.tile([P, 2, NSEQ], F32, name=f"hseg32{d}_0"),
                    const.tile([P, 2, NSEQ], F32, name=f"hseg32{d}_1"),
                ]
                nc.gpsimd.dma_start(
                    out=hseg32[d][1][:], in_=h0d.ap().rearrange("c p w -> p c w")
                )
                hseg16[d] = const.tile([P, 2, NSEQ], MDT, name=f"hseg16{d}")
                nc.vector.tensor_copy(hseg16[d][:], hseg32[d][1][:])
                outs_sb[d] = const.tile([P, T, 2, NSEQ], MDT, name=f"outs_sb{d}")
                if DUMMY:
                    nc.vector.memset(outs_sb[d][:], 0.0)

            with (
                tc.tile_pool(name="segps0", bufs=2, space="PSUM") as sps0,
                tc.tile_pool(name="segps1", bufs=2, space="PSUM") as sps1,
                tc.tile_pool(name="segew", bufs=4) as sew,
            ):
                spools = {0: sps0, 1: sps1}
                for t in range(0 if DUMMY else T):
                    for d in (0, 1):
                        lo = PAD + t if d == 0 else PAD - t
                        hrhs = hseg16[d][:] if t == 0 else outs_sb[d][:, t - 1]
                        ps = spools[d].tile([P, GC, NSEQ], F32, tag="ps")
                        for c in range(GC):
                            last_extra = c < 4 and SEG_GFOLD
                            for k in range(2):
                                nc.tensor.matmul(
                                    ps[:, c, :],
                                    lhsT=whs_sb[:, k, c * P : (c + 1) * P],
                                    rhs=hrhs[:, k, :],
                                    start=(k == 0),
                                    stop=(k == 1 and not last_extra),
                                )
                            if c < 4 and SEG_GFOLD:
                                nc.tensor.matmul(
                                    ps[:, c, :],
                                    lhsT=ident[:],
                                    rhs=G_rz[:, c, lo : lo + NSEQ],
                                    start=False,
                                    stop=True,
                                )
                        if SEG_GFOLD:
                            rz = sew.tile([P, 4, NSEQ], MDT, tag="rz")
                            nc.scalar.activation(rz[:], ps[:, 0:4, :], AF.Sigmoid)
                        else:
                            grz = sew.tile([P, 4, NSEQ], F32, tag="grz")
                            nc.vector.tensor_add(
                                grz[:], ps[:, 0:4, :], G_rz[:, :, lo : lo + NSEQ]
                            )
                            rz = sew.tile([P, 4, NSEQ], MDT, tag="rz")
                            nc.scalar.activation(rz[:], grz[:], AF.Sigmoid)
                        a = sew.tile([P, 2, NSEQ], F32, tag="a")
                        for cc in range(2):
                            nc.vector.scalar_tensor_tensor(
                                out=a[:, cc, :],
                                in0=ps[:, 4 + cc, :],
                                scalar=bhnscol_sb[:, cc : cc + 1],
                                in1=rz[:, cc, :],
                                op0=OP.add,
                                op1=OP.mult,
                            )
                        s = sew.tile([P, 2, NSEQ], F32, tag="s")
                        nc.gpsimd.tensor_add(s[:], a[:], G_n[:, :, lo : lo + NSEQ])
                        n = sew.tile([P, 2, NSEQ], F32, tag="n")
                        nc.scalar.activation(n[:], s[:], AF.Tanh)
                        hprev32 = hseg32[d][(t - 1) % 2][:]
                        hcur32 = hseg32[d][t % 2][:]
                        d1 = sew.tile([P, 2, NSEQ], F32, tag="d1")
                        nc.gpsimd.tensor_sub(d1[:], hprev32, n[:])
                        e = sew.tile([P, 2, NSEQ], F32, tag="e")
                        nc.vector.tensor_mul(e[:], d1[:], rz[:, 2:4, :])
                        nc.vector.tensor_add(hcur32, n[:], e[:])
                        nc.gpsimd.tensor_add(outs_sb[d][:, t], n[:], e[:])
                for d, eng in ((0, nc.sync), (1, nc.scalar)):
                    eng.dma_start(
                        out=outs_dram[d].ap().rearrange("t p c w -> p t c w"),
                        in_=outs_sb[d][:],
                    )
            mark("segments")

    nc.compile()
    mark("tail")
    _PROGRAM = nc
    return nc


def _prep_in_maps(tokens, emb, Wi_f, Wh_f, bi_f, bh_f, Wi_r, Wh_r, bi_r, bh_r,
                  Wi_s, Wh_s, bi_s, bh_s, h0f, h0b):
    mdt = _np_mdt()

    def gfold(bi, bh):
        v = np.concatenate([(bi + bh)[: 2 * H], bi[2 * H :]]).astype(np.float32)
        return np.ascontiguousarray(v.reshape(GC, P).T)

    def bhnrow(bh):
        return np.ascontiguousarray(
            bh[2 * H :].astype(np.float32).reshape(1, 2 * P)
        ).astype(mdt)

    common = {
        "wiT_f": np.ascontiguousarray(Wi_f.T).astype(mdt),
        "whT_f": np.ascontiguousarray(Wh_f.T).astype(mdt),
        "wiT_r": np.ascontiguousarray(Wi_r.T).astype(mdt),
        "whT_r": np.ascontiguousarray(Wh_r.T).astype(mdt),
        "gfold_f": gfold(bi_f, bh_f),
        "gfold_r": gfold(bi_r, bh_r),
        "bhncol_f": np.ascontiguousarray(bh_f[2 * H :].astype(np.float32).reshape(2, P).T),
        "bhncol_r": np.ascontiguousarray(bh_r[2 * H :].astype(np.float32).reshape(2, P).T),
        "wisT": np.ascontiguousarray(Wi_s.T).astype(mdt),
        "whsT": np.ascontiguousarray(Wh_s.T).astype(mdt),
        "gsfold": gfold(bi_s, bh_s),
        "bhnrows": np.ascontiguousarray(bh_s[2 * H :].astype(np.float32).reshape(2, P).T),
    }
    in_maps = []
    for p in range(NCORES):
        m = dict(common)
        x = emb[tokens[:, p]]  # (128, 256) host-side embedding gather (indexing)
        m["xT"] = np.ascontiguousarray(
            x.T.astype(np.float32).reshape(2, P, NSEQ)
        ).astype(mdt)
        m["h0fT"] = np.ascontiguousarray(
            h0f[:, p, :].T.astype(np.float32).reshape(2, P, NSEQ)
        )
        m["h0bT"] = np.ascontiguousarray(
            h0b[:, p, :].T.astype(np.float32).reshape(2, P, NSEQ)
        )
        in_maps.append(m)
    return in_maps


_TRI_IDX = None


def _tri_indices():
    global _TRI_IDX
    if _TRI_IDX is not None:
        return _TRI_IDX
    off = np.zeros(NSEQ, dtype=np.int64)
    for r in range(1, NSEQ):
        off[r] = off[r - 1] + (NSEQ - (r - 1))
    fk, fi, ft = [], [], []
    for i in range(NSEQ):
        L = min(T, NSEQ - i)
        t = np.arange(L)
        fk.append(off[i] + t)
        fi.append(np.full(L, i))
        ft.append(t)
    bk, bi_, bt = [], [], []
    for i in range(NSEQ):
        L = min(T, i + 1)
        t = np.arange(L)
        bk.append(off[i - t] + t)
        bi_.append(np.full(L, i))
        bt.append(t)
    _TRI_IDX = (
        off,
        np.concatenate(fk), np.concatenate(fi), np.concatenate(ft),
        np.concatenate(bk), np.concatenate(bi_), np.concatenate(bt),
    )
    return _TRI_IDX


_RUNNER = None


def _get_runner():
    """Build the sharded jit callable once and cache it (run_bass_via_pjrt
    rebuilds its closure per call, which forces a jax retrace every time)."""
    global _RUNNER
    if _RUNNER is not None:
        return _RUNNER
    import jax
    from concourse import bass2jax

    bass2jax.install_neuronx_cc_hook()
    nc = build_program()
    pname = nc.partition_id_tensor.name if nc.partition_id_tensor else None
    in_names, out_names, out_avals, zero_shapes = [], [], [], []
    for alloc in nc.m.functions[0].allocations:
        if not isinstance(alloc, mybir.MemoryLocationSet):
            continue
        name = alloc.memorylocations[0].name
        if alloc.kind == "ExternalInput":
            if name != pname:
                in_names.append(name)
        elif alloc.kind == "ExternalOutput":
            out_names.append(name)
            shape = tuple(alloc.tensor_shape)
            dtype = mybir.dt.np(alloc.dtype)
            out_avals.append(jax.core.ShapedArray(shape, dtype))
            zero_shapes.append((shape, dtype))
    n_params = len(in_names)
    all_names = in_names + out_names
    if pname is not None:
        all_names = all_names + [pname]
    donate = tuple(range(n_params, n_params + len(out_names)))

    def _body(*args):
        operands = list(args)
        if pname is not None:
            operands.append(bass2jax.partition_id_tensor())
        outs = bass2jax._bass_exec_p.bind(
            *operands,
            out_avals=tuple(out_avals),
            in_names=tuple(all_names),
            out_names=tuple(out_names),
            lowering_input_output_aliases=(),
            sim_require_finite=True,
            sim_require_nnan=True,
            nc=nc,
        )
        return tuple(outs)

    devices = jax.devices()[:NCORES]
    mesh = bass2jax.Mesh(np.asarray(devices), ("core",))
    in_specs = (bass2jax.PartitionSpec("core"),) * (n_params + len(out_names))
    out_specs = (bass2jax.PartitionSpec("core"),) * len(out_names)
    sharded = jax.jit(
        bass2jax.shard_map(
            _body, mesh=mesh, in_specs=in_specs, out_specs=out_specs, check_rep=False
        ),
        keep_unused=True,
    )
    _RUNNER = (sharded, in_names, out_names, out_avals, zero_shapes, mesh)
    return _RUNNER


_DEV_IN_CACHE = {}


def run_on_device(in_maps):
    import jax
    from jax.sharding import NamedSharding

    sharded, in_names, out_names, out_avals, zero_shapes, mesh = _get_runner()
    from jax.sharding import PartitionSpec

    spec = NamedSharding(mesh, PartitionSpec("core"))
    concat_in = [
        np.concatenate([np.asarray(in_maps[c][nm]) for c in range(NCORES)], axis=0)
        for nm in in_names
    ]
    placed = [jax.device_put(a, spec) for a in concat_in]
    zeros = _DEV_IN_CACHE.get("zeros")
    if zeros is None:
        zeros = [
            jax.device_put(np.zeros((NCORES * sh[0], *sh[1:]), dt), spec)
            for sh, dt in zero_shapes
        ]
        _DEV_IN_CACHE["zeros"] = zeros
    out_arrs = sharded(*placed, *zeros)
    results = [
        {
            nm: np.asarray(out_arrs[i]).reshape(NCORES, *out_avals[i].shape)[c]
            for i, nm in enumerate(out_names)
        }
        for c in range(NCORES)
    ]

    class _R:
        pass

    r = _R()
    r.results = results
    return r


def assemble(results):
    def _tw(arr):  # (T, P, 2, W) -> (T, W, S) with s = c*128 + p
        return arr.astype(np.float32).transpose(0, 3, 2, 1).reshape(T, NSEQ, 2 * P)
    OF = np.stack([_tw(results[p]["outs_f"]) for p in range(NCORES)])
    OB = np.stack([_tw(results[p]["outs_b"]) for p in range(NCORES)])
    _, fk, fi, ft, bk, bi_, bt = _tri_indices()
    ntri = NSEQ * (NSEQ + 1) // 2
    dp_tri = np.zeros((ntri, B, 2 * S), dtype=np.float32)
    dp_tri[fk, :, :S] = OF[:, ft, fi, :].transpose(1, 0, 2)
    dp_tri[bk, :, S:] = OB[:, bt, bi_, :].transpose(1, 0, 2)
    hid_f = OF[:, 0, NSEQ - 1, :]
    hid_b = OB[:, T - 1, NSEQ - 1, :]
    hidden = np.concatenate([hid_f, hid_b], axis=-1)[None]
    return dp_tri, hidden


def kernel(**inputs):
    in_maps = _prep_in_maps(**inputs)
    res = run_on_device(in_maps)
    return assemble(res.results)
